# revision 1
# baseline (speedup 1.0000x reference)
"""CPCLoss (CE + BDC + BEC) Trainium2 kernel.

Data-parallel over N across 8 NeuronCores (1024 rows/core).  Per row, BEC
needs the sum over ordered class pairs (j,k) of logsigmoid(x_j - x_k + eps).
With sp(z) = ln(1+e^z):   sp(d) + sp(-d) = d + 2*sp(-d)
so only the 4950 unordered pair diffs are evaluated nonlinearly; the signed
linear parts (sum of pair diffs, row sums, target-logit gathers) are exact
linear functionals the host computes in float64.  Rows are pre-sorted
descending on the host (the pair-difference multiset is permutation
invariant), making every pair diff d >= 0, hence u = exp(-d) <= 1 and
products of (1+u) stay bounded.

On device, per 128-row tile (pairs padded 4950 -> 5120 with d=0 columns
whose exact ln2 contribution the host subtracts):
  - TensorE computes all pair diffs as matmuls against a constant {+1,-1}
    difference matrix (fp16 hi/lo split of x keeps ~2^-22 input accuracy
    with exact fp32 PSUM accumulation), 512-column chunks into 2-bank PSUM
    groups, quadruple buffered; dummy matmuls during the input-DMA ramp
    hold the PE HAM clock gate at full speed.
  - ScalarE reads PSUM directly: u = exp(-d), written fp16 to SBUF.
  - VectorE: +1 in place (fp16 single-src 4x mode) then an in-place product
    fold tree (fp16 tensor_tensor, 2x mode) halving 5120 -> 640, with a
    last fp32-out level to 320 products of 16 factors (up to 2^16, beyond
    fp16 range).
  - ScalarE: ln over just those 320 products per row;  VectorE reduces.
    sum ln(1+u) = ln prod(1+u).  Exp and Ln live in one activation table
    set ('natural_log_exp_and_others', selection steered by
    _patch_act_tables), so the whole kernel does a single ACT_TABLE_LOAD.
  - The last tile instead runs ln(1+u) directly on its low half while DVE
    folds only the high half, minimizing the kernel tail.
  CE logsumexp (on host-precomputed x - rowmax) and the BEC target-row
  correction a_ln reuse the same exp/ln tables.  BDC and the second BEC
  correction differ from a_ln only by linear terms (and O(eps) wiggle far
  below fp32 noise), so the host derives them from a_ln.
"""

import math
import sys

sys.path.insert(0, "/opt/trn_rl_repo")

import numpy as np

import concourse.bacc as bacc
import concourse.tile as tile
from concourse import mybir
from concourse.bass_utils import run_bass_kernel_spmd

F32 = mybir.dt.float32
F16 = mybir.dt.float16
AF = mybir.ActivationFunctionType
ALU = mybir.AluOpType

N, C = 8192, 100
NCORES = 8
RPC = N // NCORES          # rows per core = 1024
P = 128                    # partitions
T = RPC // P               # row-tiles per core = 8
EPS = 1e-7
NPAIR = (C * (C - 1)) // 2  # 4950
NPAD = 5120                 # padded pair-columns (170 zero cols -> d=0)
CHUNK = 512
NCHUNK = NPAD // CHUNK      # 10
NGRP = 5                    # psum groups of 2 banks x 4 slots
HALF = NPAD // 2            # 2560
NFOLD = 4                   # 5120 -> 320 products of 16 (last level fp32)
NPROD = NPAD >> NFOLD       # 320
NP2 = NPROD * 2             # 640

_PAIR_J, _PAIR_K = np.triu_indices(C, 1)

_cache = {}


def _patch_act_tables():
    """Steer the activation-table allocator so Exp and Ln both resolve to the
    combined 'natural_log_exp_and_others' set (one ACT_TABLE_LOAD total,
    ~1.3us) instead of bouncing between 'exp_and_others' and 'natural_log'
    (a 1.3us reload on every switch).  Set order/length is preserved so
    act_func_set_id still indexes the real act_info.json."""
    if _cache.get("act_patched"):
        return
    from concourse.hw_specs import get_activation_tables as _real

    def _patched(arch):
        tabs = {k: set(v) for k, v in _real(arch).items()}
        for name, fns in tabs.items():
            if name != "natural_log_exp_and_others":
                fns.discard(AF.Exp)
                fns.discard(AF.Ln)
        return tabs

    bacc.get_activation_tables = _patched
    _cache["act_patched"] = True


def _build_module():
    _patch_act_tables()
    nc = bacc.Bacc("TRN2", target_bir_lowering=False, debug=False)

    xthi_d = nc.dram_tensor("xthi", [C, RPC], F16, kind="ExternalInput")
    xtlo_d = nc.dram_tensor("xtlo", [C, RPC], F16, kind="ExternalInput")
    mmat_d = nc.dram_tensor("mmat", [C, NPAD], F16, kind="ExternalInput")
    zrow_d = nc.dram_tensor("zrow", [P, T, C], F32, kind="ExternalInput")
    zsc_d = nc.dram_tensor("zsc", [P, T], F32, kind="ExternalInput")

    # parts: 0:8 sumln | 8:16 lnse | 24 a_ln | 25 sumln7b (16:24 unused)
    parts_d = nc.dram_tensor("parts", [P, 26], F32, kind="ExternalOutput")

    with tile.TileContext(nc) as tc:
        with (
            tc.tile_pool(name="consts", bufs=1) as consts,
            tc.tile_pool(name="work", bufs=3) as work,
            tc.tile_pool(name="psum", bufs=2, space="PSUM") as psum,
            tc.tile_pool(name="psum2", bufs=2, space="PSUM") as psum2,
        ):
            # ---- load inputs; spread dma_start issue across idle engines
            # so ring doorbells don't serialize on one sequencer ----
            zrow = consts.tile([P, T, C], F32)
            nc.sync.dma_start(out=zrow[:], in_=zrow_d[:])
            zsc = consts.tile([P, T], F32)
            nc.sync.dma_start(out=zsc[:], in_=zsc_d[:])
            xthi = consts.tile([C, RPC], F16)
            nc.sync.dma_start(out=xthi[:], in_=xthi_d[:])
            xtlo = consts.tile([C, RPC], F16)
            nc.sync.dma_start(out=xtlo[:], in_=xtlo_d[:])
            msb = consts.tile([C, NPAD], F16)
            for ci in range(NCHUNK):
                q0 = ci * CHUNK
                nc.sync.dma_start(
                    out=msb[:, q0:q0 + CHUNK], in_=mmat_d[:, q0:q0 + CHUNK]
                )

            # ---- accumulators / small work ----
            parts = consts.tile([P, 26], F32)
            sumln = parts[:, 0:8]
            sumln7b = parts[:, 25:26]
            lnse = parts[:, 8:16]
            aln = parts[:, 24:25]
            se = consts.tile([P, T], F32)
            zexp = consts.tile([P, T, C], F32)
            za = consts.tile([P, T, C], F32)

            # ---- a_ln prep on DVE (za = zrow - zsc = x - xy - eps) ----
            for t in range(T):
                nc.vector.tensor_scalar(
                    out=za[:, t, :], in0=zrow[:, t, :],
                    scalar1=zsc[:, t:t + 1], scalar2=None, op0=ALU.subtract,
                )

            # ---- warm the PE HAM clock gate during the input-DMA ramp ----
            dummy = consts.tile([64, 128], F16)
            nc.vector.memset(dummy[:], 0.0)
            dpt = psum.tile([P, 2, CHUNK], F32, tag="dpsum")
            for _ in range(17):
                nc.tensor.matmul(
                    out=dpt[:, 0, 0:128], lhsT=dummy[:], rhs=dummy[:],
                    start=True, stop=True,
                )

            # ---- CE + a_ln ACT work (fills ACT while first matmuls ramp) --
            nc.scalar.activation(out=zexp[:], in_=zrow[:], func=AF.Exp)
            nc.vector.tensor_reduce(
                out=se[:], in_=zexp[:], axis=mybir.AxisListType.X, op=ALU.add
            )
            nc.scalar.activation(out=za[:], in_=za[:], func=AF.Exp)
            nc.scalar.activation(out=za[:], in_=za[:], func=AF.Ln, bias=1.0)
            nc.vector.tensor_reduce(
                out=aln, in_=za[:], axis=mybir.AxisListType.XY, op=ALU.add
            )
            nc.scalar.activation(out=lnse, in_=se[:], func=AF.Ln)

            # ---- BEC hot loop ----
            def emit_ln(t, w4):
                # sum_q ln(1+u_q) = ln prod (1+u_q), folded to NPROD products
                nc.scalar.activation(out=w4[:], in_=w4[:], func=AF.Ln)
                nc.vector.tensor_reduce(
                    out=sumln[:, t:t + 1], in_=w4[:],
                    axis=mybir.AxisListType.X, op=ALU.add,
                )

            pending = None  # (t, u) awaiting its Ln
            GROUPS = [(0, 3), (3, 3), (6, 3), (9, 1)]
            for t in range(T):
                u = work.tile([P, NPAD], F16, tag="u")
                for g, (c0, nb) in enumerate(GROUPS):
                    pool_g = psum if nb == 3 else psum2
                    pt = pool_g.tile([P, nb, CHUNK], F32,
                                     tag="dpsum" if nb == 3 else "dp1")
                    for b in range(nb):
                        q0 = (c0 + b) * CHUNK
                        nc.tensor.matmul(
                            out=pt[:, b, :],
                            lhsT=xthi[:, t * P:(t + 1) * P],
                            rhs=msb[:, q0:q0 + CHUNK],
                            start=True, stop=False,
                        )
                    for b in range(nb):
                        q0 = (c0 + b) * CHUNK
                        nc.tensor.matmul(
                            out=pt[:, b, :],
                            lhsT=xtlo[:, t * P:(t + 1) * P],
                            rhs=msb[:, q0:q0 + CHUNK],
                            start=False, stop=True,
                        )
                    # u = exp(-d) straight from PSUM, as fp16
                    dst = u[:, c0 * CHUNK:(c0 + nb) * CHUNK].rearrange(
                        "p (a b) -> p a b", a=nb
                    )
                    nc.scalar.activation(
                        out=dst, in_=pt[:, :, :], func=AF.Exp, scale=-1.0
                    )
                    # interleave previous tile's ln mid-stream so ACT never
                    # stalls on this tile's fold chain
                    if g == 2 and pending is not None:
                        emit_ln(*pending)
                        pending = None
                    # v = u + 1 in place (fp16 single-src 4x mode), pipelined
                    # behind the exps; last tile's low half stays raw
                    lo = c0 * CHUNK
                    hi = (c0 + nb) * CHUNK
                    if t == T - 1:
                        lo = max(lo, HALF)
                    if hi > lo:
                        nc.vector.tensor_scalar(
                            out=u[:, lo:hi], in0=u[:, lo:hi], scalar1=1.0,
                            scalar2=None, op0=ALU.add,
                        )
                if t < T - 1:
                    # fold tree in place on DVE (fp16 2x); last level widens
                    # to fp32 (products of 16 can reach 2^16 > fp16 max)
                    sz = NPAD
                    while sz > NP2:
                        sz //= 2
                        nc.vector.tensor_tensor(
                            out=u[:, 0:sz], in0=u[:, 0:sz],
                            in1=u[:, sz:2 * sz], op=ALU.mult,
                        )
                    w4 = work.tile([P, NPROD], F32, tag="w4")
                    nc.vector.tensor_tensor(
                        out=w4[:], in0=u[:, 0:NPROD],
                        in1=u[:, NPROD:NP2], op=ALU.mult,
                    )
                    pending = (t, w4)
                else:
                    # last tile: direct ln(1+u) on the low half while DVE
                    # folds the high half — shortest kernel tail
                    nc.scalar.activation(
                        out=u[:, 0:HALF], in_=u[:, 0:HALF], func=AF.Ln,
                        bias=1.0, accum_out=sumln[:, T - 1:T],
                    )
                    sz = HALF // 2
                    while sz >= 320:
                        nc.vector.tensor_tensor(
                            out=u[:, HALF:HALF + sz], in0=u[:, HALF:HALF + sz],
                            in1=u[:, HALF + sz:HALF + 2 * sz], op=ALU.mult,
                        )
                        sz //= 2
                    w4 = work.tile([P, 320], F32, tag="w4")
                    nc.vector.tensor_copy(out=w4[:], in_=u[:, HALF:HALF + 320])
                    nc.scalar.activation(out=w4[:], in_=w4[:], func=AF.Ln)
                    nc.vector.tensor_reduce(
                        out=sumln7b, in_=w4[:],
                        axis=mybir.AxisListType.X, op=ALU.add,
                    )

            # ---- write partials ----
            nc.sync.dma_start(out=parts_d[:], in_=parts[:])

    nc.compile()
    return nc


def _get_nc():
    if "nc" not in _cache:
        _cache["nc"] = _build_module()
    return _cache["nc"]


def _build_mmat():
    m = np.zeros((C, NPAD), np.float32)
    q = np.arange(NPAIR)
    m[_PAIR_J, q] = 1.0
    m[_PAIR_K, q] = -1.0
    return m.astype(np.float16)


def _prep_core_inputs(Xs, xys, mmat_f16):
    """Xs: [RPC, C] f32 shard, rows sorted descending; xys: [RPC] f32."""
    m = Xs[:, 0:1]                    # rows sorted descending
    zrow = np.ascontiguousarray(
        (Xs - m).reshape(T, P, C).transpose(1, 0, 2)
    )  # [P, T, C]
    xt = np.ascontiguousarray(Xs.T)  # [C, RPC] f32
    xthi = xt.astype(np.float16)
    xtlo = (xt - xthi.astype(np.float32)).astype(np.float16)
    xy = np.ascontiguousarray(xys.reshape(T, P).T)  # [P, T]
    msub = np.ascontiguousarray(m[:, 0].reshape(T, P).T)  # [P, T]
    return {
        "zrow": zrow,
        "xthi": xthi,
        "xtlo": xtlo,
        "mmat": mmat_f16,
        "zsc": (xy + np.float32(EPS) - msub),
    }


def _run(X, tgt, trace=False, tmpdir=None):
    nc = _get_nc()
    mmat_f16 = _cache.get("mmat")
    if mmat_f16 is None:
        mmat_f16 = _cache["mmat"] = _build_mmat()

    xy_full = X[np.arange(N), tgt]
    # sort rows descending: pair-diff multiset is permutation invariant and
    # this guarantees d >= 0 for every (j<k) pair on device
    Xsort = np.ascontiguousarray(np.sort(X, axis=1)[:, ::-1])

    in_maps = []
    for c in range(NCORES):
        sl = slice(c * RPC, (c + 1) * RPC)
        in_maps.append(_prep_core_inputs(Xsort[sl], xy_full[sl], mmat_f16))

    res = run_bass_kernel_spmd(
        nc, in_maps, core_ids=list(range(NCORES)), trace=trace, tmpdir=tmpdir
    )

    # ---- host-side exact linear functionals (float64) ----
    X64 = np.float64(Xsort)
    xy64 = np.float64(xy_full)
    wvec = (C - 1) - 2.0 * np.arange(C, dtype=np.float64)
    sumd = (X64 @ wvec).sum()          # sum over rows of sum_{j<k}(x_j - x_k)
    xsum = X64.sum()
    xysum = xy64.sum()

    ls_eps = -math.log1p(math.exp(-EPS))
    log2 = math.log(2.0)

    sumln_tot = 0.0
    a_tot = 0.0
    mlnse_tot = 0.0
    for c in range(NCORES):
        parts = np.float64(res.results[c]["parts"])
        sumln_tot += parts[:, 0:8].sum() + parts[:, 25].sum()
        mlnse_tot += parts[:, 8:16].sum()   # lnse; row maxes added below
        a_tot += parts[:, 24].sum()

    # padded d=0 columns contribute exactly ln2 each
    sumln_tot -= N * (NPAD - NPAIR) * log2

    t_sum = a_tot
    b_sum = a_tot - (xsum - C * xysum - N * C * EPS)

    ce_sum = mlnse_tot + X64[:, 0].sum() - xysum
    s_rest = a_tot + b_sum - sumd - 2.0 * sumln_tot + N * 101 * ls_eps

    loss_ce = ce_sum / N
    loss_bdc = (t_sum - N * log2) / ((C - 1) * N)
    loss_bec = -0.5 * s_rest / ((C - 1) * (C - 2) * N)
    loss = loss_ce + loss_bdc + loss_bec
    outs = tuple(
        np.float32(v) for v in (loss, loss_ce, loss_bdc, loss_bec)
    )
    return outs, res


def kernel(inputs, targets):
    X = np.ascontiguousarray(np.asarray(inputs, dtype=np.float32))
    tgt = np.asarray(targets).astype(np.int64)
    assert X.shape == (N, C), X.shape
    outs, _ = _run(X, tgt, trace=False)
    return outs

